# revision 26
# baseline (speedup 1.0000x reference)
"""PostCrossAttention Trainium2 kernel.

Reference computation (per batch b):
    qh = (q @ Wq.T)  split into H=8 heads of dh=96   -> [H, N, 96]
    kh = (k @ Wk.T)  likewise
    vh = (v @ Wv.T)  split into H=8 heads of dv=64   -> [H, N, 64]
    S  = qh @ kh.T * SCALE          (SCALE = (256//8)**-0.5 = 32**-0.5)
    A  = softmax(S, axis=-1)
    A  = A * m / (H * sum(m, -1, keepdims))
    x  = A @ vh   -> concat heads -> [N, 512]

Sharding: 8 cores = 4 batches x 2 head-groups (4 heads each).
Each core receives host-pre-transposed fp16 operands and computes the
un-normalized numerator U^T plus the softmax denominator row; the final
division, transpose and head-concat happen on the host.

Device dataflow (per core, per head, per i-half):
    S.T[j,i] = Kp @ Qp.T             (PE, K=96 contraction, 1 jt ahead)
    e.T      = exp(S.T*SCALE - 4ln2) (ACT, from PSUM, fp16 out)
    B.T      = e.T * masks.T         (DVE, fp16)
    U.T[0:64,i]  += Vp[jt].T @ B.T[jt]   (PE, accumulated over jt)
    sum_j e: pairwise tree — pairs on DVE, quads/octs on Pool (GpSimd),
             final two accumulators contracted by ones-matmuls into
             U.T[64:65,i] (PE, PSUM-accumulated)
    U.T[0:65] -> SBUF (DVE/Pool split copy) -> DRAM (fp32)
Host: x[i, h*64+d] = U.T[d,i] / (8 * summ[i] * sumexp[i])   (numpy)
"""

import os
import sys

for _p in ("/opt/trn_rl_repo",):
    if _p not in sys.path:
        sys.path.insert(0, _p)

# NOTE: BASS_LDW_OPT=1 (walrus ldw-opt) fails codegen: "InstLdweights is
# not compatible with LDW optimization" — bass-emitted LDWs can't use it.

from contextlib import ExitStack

import numpy as np

import concourse.bass as bass
import concourse.bacc as bacc_mod
import concourse.bass_utils as _bu

# walrus's LDWEIGHTS dedup pass is off by default; repeated stationary
# reloads (ones / Vp / KpT reused across matmuls) serialize the PE here,
# so turn it on (correctness is covered by the test harness).
if not getattr(_bu, "_ldw_opt_patched", False):
    _orig_run_command = _bu.run_command

    def _run_command_ldw(argv, **kwargs):
        import os as _os
        if _os.environ.get("BASS_LDW_OPT", "0") == "1":
            argv = [a.replace("--enable-ldw-opt=false", "--enable-ldw-opt=true")
                    if isinstance(a, str) else a for a in argv]
        return _orig_run_command(argv, **kwargs)

    _bu.run_command = _run_command_ldw
    _bu._ldw_opt_patched = True
import concourse.mybir as mybir
import concourse.tile as tile

F32 = mybir.dt.float32
# PE-only operands (q/k/v/w projections) run fp16 for precision — same PE
# speed as bf16.  Everything the DVE touches (masks, exp, bsb, accumulators,
# vp) must be bf16: DVE's 2x mode engages for bf16 but NOT fp16 (measured
# 653ns vs 1030ns per [128,1024] tensor_tensor).
import ml_dtypes
DT_PE = mybir.dt.float16
DT_PE_NP = np.float16
DT_VE = mybir.dt.bfloat16
DT_VE_NP = ml_dtypes.bfloat16

# Problem constants (hardcoded per harness contract)
B, N, C, CV, H = 4, 2048, 768, 512, 8
DH, DV = C // H, CV // H          # 96, 64
NH = 4                            # heads per core
NDO = NH * DH                     # 384 projected q/k dims per core
NDV = NH * DV                     # 256 projected v dims per core
SCALE = float((256 // 8) ** (-0.5))
# shift exp into comfortable fp16 range; cancels in U/sumexp ratio
EXP_BIAS = float(-4.0 * np.log(2.0))
N_CORES = 8


def build_nc(NT: int = N):
    """Build the per-core Bass program. NT = token count (param for small sims)."""
    NJT = NT // 128               # j tiles
    assert NT % 512 == 0

    NCT = C // 128                # 6 c tiles
    NVT = CV // 128               # 4 cv tiles
    WALL = 2 * NCT * NDO + NVT * NDV
    nc = bacc_mod.Bacc()
    # all inputs host-packed to the exact SBUF image: [128, k*W] where
    # partition p row-interleaves rows {p, 128+p, ...} of the logical tensor
    qT = nc.declare_dram_parameter("qT", [128, NCT * NT], DT_PE, isOutput=False)
    kT = nc.declare_dram_parameter("kT", [128, NCT * NT], DT_PE, isOutput=False)
    vT = nc.declare_dram_parameter("vT", [128, NVT * NT], DT_PE, isOutput=False)
    mT = nc.declare_dram_parameter("mT", [128, NJT * NT], DT_VE, isOutput=False)
    wall = nc.declare_dram_parameter("wall", [128, WALL], DT_PE, isOutput=False)
    # out rows 0..63: U^T (numerator, transposed); row 64: sumexp
    out = nc.declare_dram_parameter("out", [DV + 1, NH * NT], F32, isOutput=True)

    IH = min(1024, NT)            # i-half width
    NHF = NT // IH                # number of i-halves

    with ExitStack() as top:
        tc = top.enter_context(tile.TileContext(nc))
        persist = top.enter_context(tc.tile_pool(name="persist", bufs=1))

        # masks (transposed) resident in SBUF; DMA'd on the sync ring after
        # q/k (FIFO = priority), v on the scalar ring in parallel.
        mt_all = persist.tile([128, NJT, NT], DT_VE, tag="mt", name="mt_all")
        mt_tiles = [mt_all[:, jt, :] for jt in range(NJT)]

        # ---- projections ----
        qpt = [persist.tile([DH, NT], DT_PE, tag=f"qpt{h}", name=f"qpt{h}") for h in range(NH)]
        kpt = [persist.tile([DH, NT], DT_PE, tag=f"kpt{h}", name=f"kpt{h}") for h in range(NH)]
        vp = persist.tile([128, NJT, NDV], DT_VE, tag="vp", name="vp")

        with ExitStack() as projctx:
            qkv_pool = projctx.enter_context(tc.tile_pool(name="qkv", bufs=1))
            w_pool = projctx.enter_context(tc.tile_pool(name="w", bufs=1))
            ppsum = projctx.enter_context(
                tc.tile_pool(name="ppsum", bufs=4, space="PSUM"))

            def load_whole(dram, n_tiles, width, tag, split=1, eng=None):
                t = qkv_pool.tile([128, n_tiles, width], DT_PE, tag=tag, name=tag)
                w2 = n_tiles * width
                for s in range(split):
                    a, b = s * w2 // split, (s + 1) * w2 // split
                    (eng or nc.sync).dma_start(
                        out=t.rearrange("p a n -> p (a n)")[:, a:b],
                        in_=dram[:, a:b])
                return [t[:, i, :] for i in range(n_tiles)]

            w_sb = w_pool.tile([128, WALL], DT_PE, tag="wall", name="w_sb")
            # Wq gates the first projection matmul — DMA it alone first on
            # the sync ring; Wk+Wv and v stream on the scalar ring.
            wq_end = NCT * NDO
            nc.sync.dma_start(out=w_sb[:, 0:wq_end // 2],
                              in_=wall[:, 0:wq_end // 2])
            nc.sync.dma_start(out=w_sb[:, wq_end // 2:wq_end],
                              in_=wall[:, wq_end // 2:wq_end])
            nc.scalar.dma_start(out=w_sb[:, wq_end:WALL],
                                in_=wall[:, wq_end:WALL])
            wqts = [w_sb[:, i * NDO:(i + 1) * NDO] for i in range(NCT)]
            wkts = [w_sb[:, (NCT + i) * NDO:(NCT + i + 1) * NDO]
                    for i in range(NCT)]
            wv0 = 2 * NCT * NDO
            wvts = [w_sb[:, wv0 + i * NDV:wv0 + (i + 1) * NDV]
                    for i in range(NVT)]
            qts = load_whole(qT, NCT, NT, "q", split=NCT)
            kts = load_whole(kT, NCT, NT, "k", split=NCT)
            vts = load_whole(vT, NVT, NT, "v", eng=nc.scalar)
            # masks ride the scalar ring: it is otherwise idle after v,
            # while the sync ring still streams q/k for the projections
            for s in range(4):
                a, b = s * NJT // 4, (s + 1) * NJT // 4
                nc.scalar.dma_start(
                    out=mt_all[:, a:b, :],
                    in_=mT[:, a * NT:b * NT])

            # QpT/KpT: out[dh, tok-chunk] = W_slice @ x.T
            # ci outer so 4 chunks share one stationary load per (h, ci)
            NCH = NT // 512
            for h in range(NH):
                for dst, wts, xts in ((qpt, wqts, qts), (kpt, wkts, kts)):
                    pss = [ppsum.tile([DH, 512], F32, tag="pp", name="pp")
                           for _ in range(NCH)]
                    for ci in range(NCT):
                        for ch in range(NCH):
                            nc.tensor.matmul(
                                pss[ch],
                                lhsT=wts[ci][:, h * DH:(h + 1) * DH],
                                rhs=xts[ci][:, ch * 512:(ch + 1) * 512],
                                start=(ci == 0), stop=(ci == NCT - 1),
                            )
                    for ch in range(NCH):
                        nc.vector.tensor_copy(
                            out=dst[h][:, ch * 512:(ch + 1) * 512], in_=pss[ch])

            # Vp natural: out[tok-tile, dv_all]
            for jt in range(NJT):
                ps = ppsum.tile([128, NDV], F32, tag="pv", name="pv")
                for ci in range(NVT):
                    nc.tensor.matmul(
                        ps,
                        lhsT=vts[ci][:, jt * 128:(jt + 1) * 128],
                        rhs=wvts[ci],
                        start=(ci == 0), stop=(ci == NVT - 1),
                    )
                nc.vector.tensor_copy(out=vp[:, jt, :], in_=ps)

        # ---- attention ----
        ones = persist.tile([128, 1], DT_VE, tag="ones", name="ones")
        nc.vector.memset(ones, 1.0)
        ebias = persist.tile([128, 1], F32, tag="ebias", name="ebias")
        nc.vector.memset(ebias, EXP_BIAS)

        spsum = top.enter_context(tc.tile_pool(name="spsum", bufs=3, space="PSUM"))
        utpsum = top.enter_context(tc.tile_pool(name="utpsum", bufs=1, space="PSUM"))
        streams = top.enter_context(tc.tile_pool(name="streams", bufs=3))
        accp = top.enter_context(tc.tile_pool(name="accp", bufs=3))
        utsb_pool = top.enter_context(tc.tile_pool(name="utsb", bufs=2))

        # Exp-sum: pairwise tree on DVE (2x mode) down to 4 quad
        # accumulators, contracted over partitions by accumulating
        # ones-matmuls on PE into ut_ps row DV (their own PSUM accumulation
        # group, concurrent with the A@V group in rows 0:DV; emitted in the
        # deferred tail so PE never waits on the DVE tree).
        # GpSimd/Pool compute is deliberately unused: its tensor ops hog
        # SBUF ports and slow concurrent DVE/PE ops ~4x (measured).
        #
        # A@V consumes bsb at AV_LAG j-tiles behind the S matmul: the
        # S->exp->mult->A@V chain takes ~2.3us, so with less lookahead the
        # PE stalls ~1us per jt waiting for bsb (measured). Lag costs only
        # SBUF buffers; PSUM needs are unchanged.
        AV_LAG = 5

        pending = []   # deferred phase tail, flushed early in next phase

        def emit_tail(ut_ps, quads, h, half):
            for qi, quad in enumerate(quads):
                for ic in range(IH // 512):
                    sl = slice(ic * 512, (ic + 1) * 512)
                    nc.tensor.matmul(
                        ut_ps[DV:DV + 1, sl],
                        lhsT=ones,
                        rhs=quad[:, sl],
                        start=(qi == 0), stop=(qi == len(quads) - 1),
                        skip_group_check=True,
                    )
            ut_sb = utsb_pool.tile([DV + 1, IH], F32, tag="utsb", name="ut_sb")
            # drain copy split ACT/DVE to balance the pacing engines
            # (GPSIMD cannot read PSUM)
            hw = IH // 2
            nc.scalar.copy(out=ut_sb[:, 0:hw], in_=ut_ps[0:DV + 1, 0:hw])
            nc.vector.tensor_copy(out=ut_sb[:, hw:IH],
                                  in_=ut_ps[0:DV + 1, hw:IH])
            o0 = (h * NHF + half) * IH
            nc.sync.dma_start(out=out[:, o0:o0 + IH], in_=ut_sb)

        for h in range(NH):
            for ihalf in range(NHF):
                i0 = ihalf * IH
                ut_ps = utpsum.tile([128, IH], F32, tag="ut", name="ut")

                prev_exp = None
                prev_pair = None
                quads = []
                av_queue = []

                def emit_av(jt, bsb):
                    first, last = (jt == 0), (jt == NJT - 1)
                    for ic in range(IH // 512):
                        sl = slice(ic * 512, (ic + 1) * 512)
                        nc.tensor.matmul(
                            ut_ps[0:DV, sl],
                            lhsT=vp[:, jt, h * DV:(h + 1) * DV],
                            rhs=bsb[:, sl],
                            start=first, stop=last, skip_group_check=True,
                        )

                for jt in range(NJT):
                    s_ps = spsum.tile([128, IH], F32, tag="s", name="s_ps")
                    for q2 in range(IH // 512):
                        nc.tensor.matmul(
                            s_ps[:, q2 * 512:(q2 + 1) * 512],
                            lhsT=kpt[h][:, jt * 128:(jt + 1) * 128],
                            rhs=qpt[h][:, i0 + q2 * 512: i0 + (q2 + 1) * 512],
                            start=True, stop=True,
                        )
                    expst = streams.tile([128, IH], DT_VE, tag="expst",
                                         name="expst", bufs=8)
                    nc.scalar.activation(
                        out=expst, in_=s_ps,
                        func=mybir.ActivationFunctionType.Exp, scale=SCALE,
                        bias=ebias[:, 0:1],
                    )
                    bsb = streams.tile([128, IH], DT_VE, tag="b", name="bsb",
                                       bufs=AV_LAG + 3)
                    nc.vector.tensor_tensor(
                        out=bsb, in0=expst, in1=mt_tiles[jt][:, i0:i0 + IH],
                        op=mybir.AluOpType.mult)
                    av_queue.append((jt, bsb))
                    if len(av_queue) > AV_LAG:
                        emit_av(*av_queue.pop(0))
                    if jt % 2 == 1:
                        pair = accp.tile([128, IH], DT_VE, tag="pair",
                                         name="pair", bufs=3)
                        nc.vector.tensor_tensor(
                            out=pair, in0=prev_exp, in1=expst,
                            op=mybir.AluOpType.add)
                        if prev_pair is None:
                            prev_pair = pair
                        else:
                            quad = accp.tile([128, IH], DT_VE, tag="quad",
                                             name="quad", bufs=6)
                            nc.vector.tensor_tensor(
                                out=quad, in0=prev_pair, in1=pair,
                                op=mybir.AluOpType.add)
                            prev_pair = None
                            quads.append(quad)
                    prev_exp = expst
                    # flush previous phase's tail once this phase's pipeline
                    # is primed (keeps PE from stalling on the exp-sum tree
                    # tail at the phase boundary)
                    if jt == 1 and pending:
                        emit_tail(*pending.pop())
                for jb in av_queue:
                    emit_av(*jb)
                pending.append((ut_ps, quads, h, ihalf))
                emit_tail(*pending.pop())

    nc.finalize()
    return nc


_NC_CACHE: dict = {}


def get_nc(NT: int = N):
    if NT not in _NC_CACHE:
        _NC_CACHE[NT] = build_nc(NT)
    return _NC_CACHE[NT]


def _pack(x):
    """[k*128, W] -> [128, k*W]: partition p holds rows {p, 128+p, ...}."""
    k = x.shape[0] // 128
    return x.reshape(k, 128, -1).transpose(1, 0, 2).reshape(128, -1)


def pack_core(qb, kb, vb, mb, wq_s, wk_s, wv_s):
    """Build one core's packed input dict from raw (transposed) slices.

    q/k/v/w go fp16 (PE-only); masks go bf16 (DVE operand)."""

    def f16(x):
        return np.ascontiguousarray(_pack(x.astype(np.float32).astype(DT_PE_NP)))

    wall = np.concatenate(
        [_pack(wq_s.astype(np.float32).astype(DT_PE_NP)),
         _pack(wk_s.astype(np.float32).astype(DT_PE_NP)),
         _pack(wv_s.astype(np.float32).astype(DT_PE_NP))], axis=1)
    mbf = np.ascontiguousarray(_pack(mb.astype(np.float32).astype(DT_VE_NP)))
    return {
        "qT": f16(qb), "kT": f16(kb), "vT": f16(vb), "mT": mbf,
        "wall": np.ascontiguousarray(wall),
    }


def make_in_maps(q, k, v, masks, Wq, Wk, Wv):
    """Host-side shard + layout prep. Returns per-core input dicts."""
    in_maps = []
    for c in range(N_CORES):
        b, hg = c // 2, c % 2
        in_maps.append(pack_core(
            q[b].T, k[b].T, v[b].T, masks[b].T,
            Wq[hg * NDO:(hg + 1) * NDO, :].T,
            Wk[hg * NDO:(hg + 1) * NDO, :].T,
            Wv[hg * NDV:(hg + 1) * NDV, :].T,
        ))
    return in_maps


def postprocess(res_out, masks_b, NT=N):
    """Host epilogue for one core: [65, NH*NT] raw -> [NT, NDV] slice.

    res_out rows 0..63: U^T numerator; row 64: sumexp (both carry the
    same exp bias factor, which cancels in the ratio).
    x[i, h*DV+d] = U^T[d, i] / (H * summ[i] * sumexp[i])
    """
    IH = min(1024, NT)
    NHF = NT // IH
    arr = np.asarray(res_out, np.float32).reshape(DV + 1, NH, NHF, IH)
    U = arr[:DV]                       # [DV, NH, NHF, IH]
    se = arr[DV]                       # [NH, NHF, IH]
    summ = masks_b.astype(np.float64).sum(axis=1).astype(np.float32)
    den = float(H) * summ.reshape(NHF, IH)[None, :, :] * se
    xh = U / den[None]                 # [DV, NH, NHF, IH]
    return xh.transpose(2, 3, 1, 0).reshape(NT, NH * DV)


def _reset_device():
    import ctypes
    try:
        lib = ctypes.CDLL("/opt/axon/libaxon_pjrt.so")
        lib.axon_reset.restype = ctypes.c_int64
        lib.axon_reset()
    except Exception:
        pass


def kernel(q, k, v, masks, Wq, Wk, Wv, **_unused):
    from concourse.bass_utils import run_bass_kernel_spmd

    q, k, v, masks = (np.asarray(x) for x in (q, k, v, masks))
    Wq, Wk, Wv = (np.asarray(x) for x in (Wq, Wk, Wv))

    nc = get_nc(N)
    in_maps = make_in_maps(q, k, v, masks, Wq, Wk, Wv)
    try:
        res = run_bass_kernel_spmd(
            nc, in_maps, core_ids=list(range(N_CORES))).results
    except Exception:
        # wedged accelerator (e.g. NRT_EXEC_UNIT_UNRECOVERABLE) — reset + retry
        _reset_device()
        res = run_bass_kernel_spmd(
            nc, in_maps, core_ids=list(range(N_CORES))).results

    full = np.empty((B, N, CV), np.float32)
    for c in range(N_CORES):
        b, hg = c // 2, c % 2
        full[b][:, hg * NDV:(hg + 1) * NDV] = postprocess(
            res[c]["out"], masks[b])
    return full


# revision 27
# speedup vs baseline: 1.1100x; 1.1100x over previous
"""PostCrossAttention Trainium2 kernel.

Reference computation (per batch b):
    qh = (q @ Wq.T)  split into H=8 heads of dh=96   -> [H, N, 96]
    kh = (k @ Wk.T)  likewise
    vh = (v @ Wv.T)  split into H=8 heads of dv=64   -> [H, N, 64]
    S  = qh @ kh.T * SCALE          (SCALE = (256//8)**-0.5 = 32**-0.5)
    A  = softmax(S, axis=-1)
    A  = A * m / (H * sum(m, -1, keepdims))
    x  = A @ vh   -> concat heads -> [N, 512]

Sharding: 8 cores = 4 batches x 2 head-groups (4 heads each).
Each core receives host-pre-transposed fp16 operands and computes the
un-normalized numerator U^T plus the softmax denominator row; the final
division, transpose and head-concat happen on the host.

Device dataflow (per core, per head, per i-half):
    S.T[j,i] = Kp @ Qp.T             (PE, K=96 contraction, 1 jt ahead)
    e.T      = exp(S.T*SCALE - 4ln2) (ACT, from PSUM, fp16 out)
    B.T      = e.T * masks.T         (DVE, fp16)
    U.T[0:64,i]  += Vp[jt].T @ B.T[jt]   (PE, accumulated over jt)
    sum_j e: pairwise tree — pairs on DVE, quads/octs on Pool (GpSimd),
             final two accumulators contracted by ones-matmuls into
             U.T[64:65,i] (PE, PSUM-accumulated)
    U.T[0:65] -> SBUF (DVE/Pool split copy) -> DRAM (fp32)
Host: x[i, h*64+d] = U.T[d,i] / (8 * summ[i] * sumexp[i])   (numpy)
"""

import os
import sys

for _p in ("/opt/trn_rl_repo",):
    if _p not in sys.path:
        sys.path.insert(0, _p)

# NOTE: BASS_LDW_OPT=1 (walrus ldw-opt) fails codegen: "InstLdweights is
# not compatible with LDW optimization" — bass-emitted LDWs can't use it.

from contextlib import ExitStack

import numpy as np

import concourse.bass as bass
import concourse.bacc as bacc_mod
import concourse.bass_utils as _bu

# walrus's LDWEIGHTS dedup pass is off by default; repeated stationary
# reloads (ones / Vp / KpT reused across matmuls) serialize the PE here,
# so turn it on (correctness is covered by the test harness).
if not getattr(_bu, "_ldw_opt_patched", False):
    _orig_run_command = _bu.run_command

    def _run_command_ldw(argv, **kwargs):
        import os as _os
        if _os.environ.get("BASS_LDW_OPT", "0") == "1":
            argv = [a.replace("--enable-ldw-opt=false", "--enable-ldw-opt=true")
                    if isinstance(a, str) else a for a in argv]
        return _orig_run_command(argv, **kwargs)

    _bu.run_command = _run_command_ldw
    _bu._ldw_opt_patched = True
import concourse.mybir as mybir
import concourse.tile as tile

F32 = mybir.dt.float32
# PE-only operands (q/k/v/w projections) run fp16 for precision — same PE
# speed as bf16.  Everything the DVE touches (masks, exp, bsb, accumulators,
# vp) must be bf16: DVE's 2x mode engages for bf16 but NOT fp16 (measured
# 653ns vs 1030ns per [128,1024] tensor_tensor).
import ml_dtypes
DT_PE = mybir.dt.float16
DT_PE_NP = np.float16
DT_VE = mybir.dt.bfloat16
DT_VE_NP = ml_dtypes.bfloat16

# Problem constants (hardcoded per harness contract)
B, N, C, CV, H = 4, 2048, 768, 512, 8
DH, DV = C // H, CV // H          # 96, 64
NH = 4                            # heads per core
NDO = NH * DH                     # 384 projected q/k dims per core
NDV = NH * DV                     # 256 projected v dims per core
SCALE = float((256 // 8) ** (-0.5))
# shift exp into comfortable fp16 range; cancels in U/sumexp ratio
EXP_BIAS = float(-4.0 * np.log(2.0))
N_CORES = 8


def build_nc(NT: int = N):
    """Build the per-core Bass program. NT = token count (param for small sims)."""
    NJT = NT // 128               # j tiles
    assert NT % 512 == 0

    NCT = C // 128                # 6 c tiles
    NVT = CV // 128               # 4 cv tiles
    WALL = 2 * NCT * NDO + NVT * NDV
    nc = bacc_mod.Bacc()
    # all inputs host-packed to the exact SBUF image: [128, k*W] where
    # partition p row-interleaves rows {p, 128+p, ...} of the logical tensor
    qT = nc.declare_dram_parameter("qT", [128, NCT * NT], DT_PE, isOutput=False)
    kT = nc.declare_dram_parameter("kT", [128, NCT * NT], DT_PE, isOutput=False)
    vT = nc.declare_dram_parameter("vT", [128, NVT * NT], DT_PE, isOutput=False)
    mT = nc.declare_dram_parameter("mT", [128, NJT * NT], DT_VE, isOutput=False)
    wall = nc.declare_dram_parameter("wall", [128, WALL], DT_PE, isOutput=False)
    # out rows 0..63: U^T (numerator, transposed); row 64: sumexp
    out = nc.declare_dram_parameter("out", [DV + 1, NH * NT], F32, isOutput=True)

    IH = min(1024, NT)            # i-half width
    NHF = NT // IH                # number of i-halves

    with ExitStack() as top:
        tc = top.enter_context(tile.TileContext(nc))
        persist = top.enter_context(tc.tile_pool(name="persist", bufs=1))

        # masks (transposed) resident in SBUF; DMA'd on the sync ring after
        # q/k (FIFO = priority), v on the scalar ring in parallel.
        mt_all = persist.tile([128, NJT, NT], DT_VE, tag="mt", name="mt_all")
        mt_tiles = [mt_all[:, jt, :] for jt in range(NJT)]

        # ---- projections ----
        qpt = [persist.tile([DH, NT], DT_PE, tag=f"qpt{h}", name=f"qpt{h}") for h in range(NH)]
        kpt = [persist.tile([DH, NT], DT_PE, tag=f"kpt{h}", name=f"kpt{h}") for h in range(NH)]
        vp = persist.tile([128, NJT, NDV], DT_VE, tag="vp", name="vp")

        with ExitStack() as projctx:
            qkv_pool = projctx.enter_context(tc.tile_pool(name="qkv", bufs=1))
            w_pool = projctx.enter_context(tc.tile_pool(name="w", bufs=1))
            ppsum = projctx.enter_context(
                tc.tile_pool(name="ppsum", bufs=4, space="PSUM"))

            def load_whole(dram, n_tiles, width, tag, split=1, eng=None):
                t = qkv_pool.tile([128, n_tiles, width], DT_PE, tag=tag, name=tag)
                w2 = n_tiles * width
                for s in range(split):
                    a, b = s * w2 // split, (s + 1) * w2 // split
                    (eng or nc.sync).dma_start(
                        out=t.rearrange("p a n -> p (a n)")[:, a:b],
                        in_=dram[:, a:b])
                return [t[:, i, :] for i in range(n_tiles)]

            w_sb = w_pool.tile([128, WALL], DT_PE, tag="wall", name="w_sb")
            # Wq gates the first projection matmul — DMA it alone first on
            # the sync ring; Wk+Wv and v stream on the scalar ring.
            wq_end = NCT * NDO
            nc.sync.dma_start(out=w_sb[:, 0:wq_end // 2],
                              in_=wall[:, 0:wq_end // 2])
            nc.sync.dma_start(out=w_sb[:, wq_end // 2:wq_end],
                              in_=wall[:, wq_end // 2:wq_end])
            nc.scalar.dma_start(out=w_sb[:, wq_end:WALL],
                                in_=wall[:, wq_end:WALL])
            wqts = [w_sb[:, i * NDO:(i + 1) * NDO] for i in range(NCT)]
            wkts = [w_sb[:, (NCT + i) * NDO:(NCT + i + 1) * NDO]
                    for i in range(NCT)]
            wv0 = 2 * NCT * NDO
            wvts = [w_sb[:, wv0 + i * NDV:wv0 + (i + 1) * NDV]
                    for i in range(NVT)]
            qts = load_whole(qT, NCT, NT, "q", split=NCT)
            kts = load_whole(kT, NCT, NT, "k", split=NCT)
            vts = load_whole(vT, NVT, NT, "v", eng=nc.scalar)
            for s in range(4):
                a, b = s * NJT // 4, (s + 1) * NJT // 4
                nc.sync.dma_start(
                    out=mt_all[:, a:b, :],
                    in_=mT[:, a * NT:b * NT])

            # QpT/KpT: out[dh, tok-chunk] = W_slice @ x.T
            # ci outer so 4 chunks share one stationary load per (h, ci)
            NCH = NT // 512
            for h in range(NH):
                for dst, wts, xts in ((qpt, wqts, qts), (kpt, wkts, kts)):
                    pss = [ppsum.tile([DH, 512], F32, tag="pp", name="pp")
                           for _ in range(NCH)]
                    for ci in range(NCT):
                        for ch in range(NCH):
                            nc.tensor.matmul(
                                pss[ch],
                                lhsT=wts[ci][:, h * DH:(h + 1) * DH],
                                rhs=xts[ci][:, ch * 512:(ch + 1) * 512],
                                start=(ci == 0), stop=(ci == NCT - 1),
                            )
                    for ch in range(NCH):
                        nc.vector.tensor_copy(
                            out=dst[h][:, ch * 512:(ch + 1) * 512], in_=pss[ch])

            # Vp natural: out[tok-tile, dv_all]
            for jt in range(NJT):
                ps = ppsum.tile([128, NDV], F32, tag="pv", name="pv")
                for ci in range(NVT):
                    nc.tensor.matmul(
                        ps,
                        lhsT=vts[ci][:, jt * 128:(jt + 1) * 128],
                        rhs=wvts[ci],
                        start=(ci == 0), stop=(ci == NVT - 1),
                    )
                nc.vector.tensor_copy(out=vp[:, jt, :], in_=ps)

        # ---- attention ----
        ones = persist.tile([128, 1], DT_VE, tag="ones", name="ones")
        nc.vector.memset(ones, 1.0)
        ebias = persist.tile([128, 1], F32, tag="ebias", name="ebias")
        nc.vector.memset(ebias, EXP_BIAS)

        spsum = top.enter_context(tc.tile_pool(name="spsum", bufs=3, space="PSUM"))
        utpsum = top.enter_context(tc.tile_pool(name="utpsum", bufs=1, space="PSUM"))
        streams = top.enter_context(tc.tile_pool(name="streams", bufs=3))
        accp = top.enter_context(tc.tile_pool(name="accp", bufs=3))
        utsb_pool = top.enter_context(tc.tile_pool(name="utsb", bufs=2))

        # Exp-sum: pairwise tree on DVE (2x mode) down to 4 quad
        # accumulators, contracted over partitions by accumulating
        # ones-matmuls on PE into ut_ps row DV (their own PSUM accumulation
        # group, concurrent with the A@V group in rows 0:DV; emitted in the
        # deferred tail so PE never waits on the DVE tree).
        # GpSimd/Pool compute is deliberately unused: its tensor ops hog
        # SBUF ports and slow concurrent DVE/PE ops ~4x (measured).
        #
        # A@V consumes bsb at AV_LAG j-tiles behind the S matmul: the
        # S->exp->mult->A@V chain takes ~2.3us, so with less lookahead the
        # PE stalls ~1us per jt waiting for bsb (measured). Lag costs only
        # SBUF buffers; PSUM needs are unchanged.
        AV_LAG = 5

        pending = []   # deferred phase tail, flushed early in next phase

        def emit_tail(ut_ps, quads, h, half):
            for qi, quad in enumerate(quads):
                for ic in range(IH // 512):
                    sl = slice(ic * 512, (ic + 1) * 512)
                    nc.tensor.matmul(
                        ut_ps[DV:DV + 1, sl],
                        lhsT=ones,
                        rhs=quad[:, sl],
                        start=(qi == 0), stop=(qi == len(quads) - 1),
                        skip_group_check=True,
                    )
            ut_sb = utsb_pool.tile([DV + 1, IH], F32, tag="utsb", name="ut_sb")
            # drain copy split ACT/DVE to balance the pacing engines
            # (GPSIMD cannot read PSUM)
            hw = IH // 2
            nc.scalar.copy(out=ut_sb[:, 0:hw], in_=ut_ps[0:DV + 1, 0:hw])
            nc.vector.tensor_copy(out=ut_sb[:, hw:IH],
                                  in_=ut_ps[0:DV + 1, hw:IH])
            o0 = (h * NHF + half) * IH
            nc.sync.dma_start(out=out[:, o0:o0 + IH], in_=ut_sb)

        for h in range(NH):
            for ihalf in range(NHF):
                i0 = ihalf * IH
                ut_ps = utpsum.tile([128, IH], F32, tag="ut", name="ut")

                prev_exp = None
                prev_pair = None
                quads = []
                av_queue = []

                def emit_av(jt, bsb):
                    first, last = (jt == 0), (jt == NJT - 1)
                    for ic in range(IH // 512):
                        sl = slice(ic * 512, (ic + 1) * 512)
                        nc.tensor.matmul(
                            ut_ps[0:DV, sl],
                            lhsT=vp[:, jt, h * DV:(h + 1) * DV],
                            rhs=bsb[:, sl],
                            start=first, stop=last, skip_group_check=True,
                        )

                for jt in range(NJT):
                    s_ps = spsum.tile([128, IH], F32, tag="s", name="s_ps")
                    for q2 in range(IH // 512):
                        nc.tensor.matmul(
                            s_ps[:, q2 * 512:(q2 + 1) * 512],
                            lhsT=kpt[h][:, jt * 128:(jt + 1) * 128],
                            rhs=qpt[h][:, i0 + q2 * 512: i0 + (q2 + 1) * 512],
                            start=True, stop=True,
                        )
                    expst = streams.tile([128, IH], DT_VE, tag="expst",
                                         name="expst", bufs=8)
                    nc.scalar.activation(
                        out=expst, in_=s_ps,
                        func=mybir.ActivationFunctionType.Exp, scale=SCALE,
                        bias=ebias[:, 0:1],
                    )
                    bsb = streams.tile([128, IH], DT_VE, tag="b", name="bsb",
                                       bufs=AV_LAG + 3)
                    nc.vector.tensor_tensor(
                        out=bsb, in0=expst, in1=mt_tiles[jt][:, i0:i0 + IH],
                        op=mybir.AluOpType.mult)
                    av_queue.append((jt, bsb))
                    if len(av_queue) > AV_LAG:
                        emit_av(*av_queue.pop(0))
                    if jt % 2 == 1:
                        pair = accp.tile([128, IH], DT_VE, tag="pair",
                                         name="pair", bufs=3)
                        nc.vector.tensor_tensor(
                            out=pair, in0=prev_exp, in1=expst,
                            op=mybir.AluOpType.add)
                        if prev_pair is None:
                            prev_pair = pair
                        else:
                            quad = accp.tile([128, IH], DT_VE, tag="quad",
                                             name="quad", bufs=6)
                            nc.vector.tensor_tensor(
                                out=quad, in0=prev_pair, in1=pair,
                                op=mybir.AluOpType.add)
                            prev_pair = None
                            quads.append(quad)
                    prev_exp = expst
                    # flush previous phase's tail once this phase's pipeline
                    # is primed (keeps PE from stalling on the exp-sum tree
                    # tail at the phase boundary)
                    if jt == 1 and pending:
                        emit_tail(*pending.pop())
                for jb in av_queue:
                    emit_av(*jb)
                pending.append((ut_ps, quads, h, ihalf))
                emit_tail(*pending.pop())

    nc.finalize()
    return nc


_NC_CACHE: dict = {}


def get_nc(NT: int = N):
    if NT not in _NC_CACHE:
        _NC_CACHE[NT] = build_nc(NT)
    return _NC_CACHE[NT]


def _pack(x):
    """[k*128, W] -> [128, k*W]: partition p holds rows {p, 128+p, ...}."""
    k = x.shape[0] // 128
    return x.reshape(k, 128, -1).transpose(1, 0, 2).reshape(128, -1)


def pack_core(qb, kb, vb, mb, wq_s, wk_s, wv_s):
    """Build one core's packed input dict from raw (transposed) slices.

    q/k/v/w go fp16 (PE-only); masks go bf16 (DVE operand)."""

    def f16(x):
        return np.ascontiguousarray(_pack(x.astype(np.float32).astype(DT_PE_NP)))

    wall = np.concatenate(
        [_pack(wq_s.astype(np.float32).astype(DT_PE_NP)),
         _pack(wk_s.astype(np.float32).astype(DT_PE_NP)),
         _pack(wv_s.astype(np.float32).astype(DT_PE_NP))], axis=1)
    mbf = np.ascontiguousarray(_pack(mb.astype(np.float32).astype(DT_VE_NP)))
    return {
        "qT": f16(qb), "kT": f16(kb), "vT": f16(vb), "mT": mbf,
        "wall": np.ascontiguousarray(wall),
    }


def make_in_maps(q, k, v, masks, Wq, Wk, Wv):
    """Host-side shard + layout prep. Returns per-core input dicts."""
    in_maps = []
    for c in range(N_CORES):
        b, hg = c // 2, c % 2
        in_maps.append(pack_core(
            q[b].T, k[b].T, v[b].T, masks[b].T,
            Wq[hg * NDO:(hg + 1) * NDO, :].T,
            Wk[hg * NDO:(hg + 1) * NDO, :].T,
            Wv[hg * NDV:(hg + 1) * NDV, :].T,
        ))
    return in_maps


def postprocess(res_out, masks_b, NT=N):
    """Host epilogue for one core: [65, NH*NT] raw -> [NT, NDV] slice.

    res_out rows 0..63: U^T numerator; row 64: sumexp (both carry the
    same exp bias factor, which cancels in the ratio).
    x[i, h*DV+d] = U^T[d, i] / (H * summ[i] * sumexp[i])
    """
    IH = min(1024, NT)
    NHF = NT // IH
    arr = np.asarray(res_out, np.float32).reshape(DV + 1, NH, NHF, IH)
    U = arr[:DV]                       # [DV, NH, NHF, IH]
    se = arr[DV]                       # [NH, NHF, IH]
    summ = masks_b.astype(np.float64).sum(axis=1).astype(np.float32)
    den = float(H) * summ.reshape(NHF, IH)[None, :, :] * se
    xh = U / den[None]                 # [DV, NH, NHF, IH]
    return xh.transpose(2, 3, 1, 0).reshape(NT, NH * DV)


def _reset_device():
    import ctypes
    try:
        lib = ctypes.CDLL("/opt/axon/libaxon_pjrt.so")
        lib.axon_reset.restype = ctypes.c_int64
        lib.axon_reset()
    except Exception:
        pass


def kernel(q, k, v, masks, Wq, Wk, Wv, **_unused):
    from concourse.bass_utils import run_bass_kernel_spmd

    q, k, v, masks = (np.asarray(x) for x in (q, k, v, masks))
    Wq, Wk, Wv = (np.asarray(x) for x in (Wq, Wk, Wv))

    nc = get_nc(N)
    in_maps = make_in_maps(q, k, v, masks, Wq, Wk, Wv)
    try:
        res = run_bass_kernel_spmd(
            nc, in_maps, core_ids=list(range(N_CORES))).results
    except Exception:
        # wedged accelerator (e.g. NRT_EXEC_UNIT_UNRECOVERABLE) — reset + retry
        _reset_device()
        res = run_bass_kernel_spmd(
            nc, in_maps, core_ids=list(range(N_CORES))).results

    full = np.empty((B, N, CV), np.float32)
    for c in range(N_CORES):
        b, hg = c // 2, c % 2
        full[b][:, hg * NDV:(hg + 1) * NDV] = postprocess(
            res[c]["out"], masks[b])
    return full
